# revision 69
# baseline (speedup 1.0000x reference)
"""CrossAttention Trainium2 kernel (Bass/Tile), batch-parallel over 8 NeuronCores.

Problem (per batch b of 8):
    x   [512, 32, 32]  -> X   [C=512, N=1024]
    ctx [512, 32, 32]  -> CTX [C=512, M=1024]
    q = Wq@X * s + bq*s ; k = Wk@CTX + bk ; v = Wv@CTX (+ bv, folded)
    per head h (8 heads x 64): simT[j,i] = sum_d k[d,j] q[d,i]
    attn = softmax_j(sim);  out[i,d] = sum_j attn[i,j] v[d,j]
    final = Wo@out + bo'          (bo' = Wo@bv + bo, folded host-side — exact,
                                   since attn weights sum to 1)

Design (per core = one batch), tuned against the TimelineSim cost model:
  - fp16 storage everywhere (same matmul cost as bf16, ~8x less rounding;
    rel err ~9e-4 vs ~7e-3 for bf16)
  - 16 iterations (i-half, head-pair, head): per iteration 8 sim matmuls
    [128j, 512i] (K=64) through FOUR 2-bank PSUM groups rotating over THREE
    pools — the 3-pool rotation removes any intra-iteration exp->sim
    coupling, so the steady state is paced purely by ACT's exp throughput
  - attn@v runs TRANSPOSED: lhsT = es[j, i-chunk(128)], rhs = vte[j, 65]
    (v columns + ones column) -> ot[i(128), 65] accumulated over 8 j-chunks
    in one PSUM bank (slice-wise accumulation group). Model cost is charged
    on output free size, so this halves attn@v PE time; the ones column
    lands the softmax denominator in the same partition as its row, making
    normalization a free-axis broadcast multiply on DVE (no partition
    broadcast). Attention waves lag their iteration by ~3 and self-chase
    exp groups via subtile deps.
  - the normalized output oallT[i, hd] flips to [hd, i] for the output
    projection via one xbar DMA transpose per (i-half, pair) on the idle
    DMA engines; late pairs use PE-transposes (identity matmul) instead to
    dodge the ~3us DMA latency at the tail
  - DMA transfers serialize on the modeled DMA fleet (~0.36 B/ns), so
    loads are ordered strictly by first need (wq/wk out-chunk 0 split off,
    x/ctx interleaved, remaining weights behind a Pool-queue spacer);
    biases are packed into one small tensor; ~230 tiny warmup matmuls keep
    the PE p-state ramp warm until real data lands
  - tail: the last o-proj groups pre-open (bias seeded into PSUM via a
    rank-1 matmul) so only their last-pair contribution waits on the final
    attention wave; fins fan out across ACT and DVE

Baseline (previous session): 112907 ns, rel err 6.8e-3.
This version (same TimelineSim metric): 93972 ns, rel err ~9e-4
(output stored fp16 and upcast on host — saves half the serialized tail
store bytes at ~5e-5 extra error).
"""

import contextlib
import sys

sys.path.insert(0, "/opt/trn_rl_repo")

import numpy as np

import concourse.bass as bass
import concourse.tile as tile
from concourse import bacc, mybir

B = 8
HEADS = 8
DH = 64
C = 512
NTOK = 1024  # 32*32
P = 128
CCH = C // P  # 4 channel chunks
JCH = NTOK // P  # 8 context-token chunks (partition dim of simT)
ICH = 2  # query-token halves of 512 (free dim)
F = 512
SCALE = DH ** (-0.5)

F16 = mybir.dt.float16
F32 = mybir.dt.float32
NPF16 = np.float16


def build_nc(reps: int = 1):
    nc = bacc.Bacc("TRN2", target_bir_lowering=False, debug=False)

    x_d = nc.dram_tensor("x", [C, NTOK], F16, kind="ExternalInput")
    c_d = nc.dram_tensor("ctx", [C, NTOK], F16, kind="ExternalInput")
    wqt_d = nc.dram_tensor("wqt", [C, C], F16, kind="ExternalInput")
    wkt_d = nc.dram_tensor("wkt", [C, C], F16, kind="ExternalInput")
    wvt_d = nc.dram_tensor("wvt", [C, C], F16, kind="ExternalInput")
    wot_d = nc.dram_tensor("wot", [C, C], F16, kind="ExternalInput")
    # biases packed host-side: bias_pack[p, oc*3 + t], t in (bq, bk, bo)
    bias_d = nc.dram_tensor("bias_pack", [P, CCH * 3], F32, kind="ExternalInput")
    borow_d = nc.dram_tensor("bo_row", [1, C], F16, kind="ExternalInput")
    ident_d = nc.dram_tensor("ident", [P, P], F16, kind="ExternalInput")
    out_d = nc.dram_tensor("out", [C, NTOK], F16, kind="ExternalOutput")

    with tile.TileContext(nc) as tc:
        with (
            tc.tile_pool(name="consts", bufs=1) as consts,
            tc.tile_pool(name="acts", bufs=1) as acts,
            tc.tile_pool(name="expp", bufs=6) as expp,
            tc.tile_pool(name="sbcp", bufs=4) as sbcp,
            tc.tile_pool(name="finp", bufs=4) as finp,
            tc.tile_pool(name="simA", bufs=1, space="PSUM") as simA,
            tc.tile_pool(name="simB", bufs=1, space="PSUM") as simB,
            tc.tile_pool(name="simC", bufs=1, space="PSUM") as simC,
            tc.tile_pool(name="otp", bufs=1, space="PSUM") as otp,
            tc.tile_pool(name="mxps", bufs=1, space="PSUM") as mxps,
        ):
          with (tc.For_i(0, reps, 1) if reps > 1 else contextlib.nullcontext()) as _i:
            # ---- constants (weights on the gpsimd SWDGE queue, one DMA per
            # weight — the 994ns SWDGE desc-gen is serial on Pool, so fewer
            # DMAs = weights land much earlier; biases ride the ACT queue,
            # which is idle until the first exp) ----
            wq_sb = consts.tile([P, CCH, C], F16, tag="wq")
            wk_sb = consts.tile([P, CCH, C], F16, tag="wk")
            wv_sb = consts.tile([P, CCH, C], F16, tag="wv")
            wo_sb = consts.tile([P, CCH, C], F16, tag="wo")
            bias_sb = consts.tile([P, CCH * 3], F32, tag="bias")
            bq_sb = bias_sb.rearrange("p (c t) -> p c t", t=3)[:, :, 0]
            bk_sb = bias_sb.rearrange("p (c t) -> p c t", t=3)[:, :, 1]
            bo_sb = bias_sb.rearrange("p (c t) -> p c t", t=3)[:, :, 2]
            ident_sb = consts.tile([P, P], F16, tag="ident")
            borow_sb = consts.tile([1, C], F16, tag="borow")
            ones_sb = consts.tile([1, F], F16, tag="ones")

            # ---- activations. DMA transfers SERIALIZE on the DMA fleet in
            # the cost model (~0.36 B/ns effective), so order strictly by
            # first need: the first sim needs q00 (x + wq[:,0:128]) and k00
            # (ctx + wk[:,0:128]). Weight out-chunk 0 is split off so those
            # prerequisites are minimal; x/ctx interleave.
            x_sb = acts.tile([P, CCH, NTOK], F16, tag="x")
            c_sb = acts.tile([P, CCH, NTOK], F16, tag="c")
            xr = x_d.rearrange("(c p) n -> p c n", p=P)
            cr = c_d.rearrange("(c p) n -> p c n", p=P)
            nc.scalar.dma_start(out=bias_sb[:, :], in_=bias_d[:, :])
            wqr = wqt_d.rearrange("(c p) o -> p c o", p=P)
            wkr = wkt_d.rearrange("(c p) o -> p c o", p=P)
            nc.gpsimd.dma_start(out=wq_sb[:, :, 0:P], in_=wqr[:, :, 0:P])
            nc.sync.dma_start(out=x_sb[:, 0:2, :], in_=xr[:, 0:2, :])
            nc.gpsimd.dma_start(out=wk_sb[:, :, 0:P], in_=wkr[:, :, 0:P])
            nc.sync.dma_start(out=c_sb[:, 0:1, :], in_=cr[:, 0:1, :])
            nc.sync.dma_start(out=x_sb[:, 2:4, :], in_=xr[:, 2:4, :])
            nc.sync.dma_start(out=c_sb[:, 1:2, :], in_=cr[:, 1:2, :])
            nc.sync.dma_start(out=c_sb[:, 2:3, :], in_=cr[:, 2:3, :])
            nc.sync.dma_start(out=c_sb[:, 3:4, :], in_=cr[:, 3:4, :])
            # spacer: delay the non-critical weight desc-gens so their
            # transfers queue behind the critical x/ctx loads on the
            # serialized DMA fleet
            spacer = consts.tile([1, 2600], F32, tag="spacer")
            nc.gpsimd.memset(spacer[:, :], 0.0)
            nc.gpsimd.dma_start(out=wq_sb[:, :, P:C], in_=wqr[:, :, P:C])
            nc.gpsimd.dma_start(out=wk_sb[:, :, P:C], in_=wkr[:, :, P:C])
            nc.gpsimd.dma_start(
                out=wv_sb[:, :, :], in_=wvt_d.rearrange("(c p) o -> p c o", p=P)
            )
            nc.gpsimd.dma_start(
                out=wo_sb[:, :, :], in_=wot_d.rearrange("(c p) o -> p c o", p=P)
            )
            nc.sync.dma_start(out=ident_sb[:, :], in_=ident_d[:, :])

            q_sb = acts.tile([P, CCH, NTOK], F16, tag="q")
            k_sb = acts.tile([P, CCH, NTOK], F16, tag="k")
            # vT with a ones column per head: [j-part, j-chunk, head, 64+1]
            vte_sb = acts.tile([P, JCH, HEADS, DH + 1], F16, tag="vte")
            nc.vector.memset(vte_sb[:, :, :, DH : DH + 1], 1.0)

            # PE p-state warmup: ~56 garbage matmuls on the ones column span
            # the first ~3us so real matmuls start at full clock
            wu = mxps.tile([P, 2 * DH], F32, tag="mx", name="warmup")
            for _ in range(230):
                nc.tensor.matmul(
                    wu[0:1, 0:DH],
                    vte_sb[0:1, 0, 0, DH : DH + 1],
                    vte_sb[0:1, :, :, DH].rearrange("p a b -> p (a b)"),
                    start=True,
                    stop=True,
                )

            # normalized attention output, [i-part, slot(=i/128, 8 slots), hd-of-pair]
            # and its transpose, [hd-of-pair, slot, i-low], per head pair
            NSLOT = NTOK // P  # 8
            oallT = [
                acts.tile([P, NSLOT, P], F16, tag=f"oallT{pr}", name=f"oallT{pr}")
                for pr in range(CCH)
            ]
            oallP = [
                acts.tile([P, NSLOT, P], F16, tag=f"oallP{pr}", name=f"oallP{pr}")
                for pr in range(CCH)
            ]

            # ---- projection groups; pool chosen by caller (the sim PSUM
            # pools are free before the attention loop starts) ----
            def emit_v_group(mc, pool=None, tag="mx"):
                pool = pool if pool is not None else mxps
                ps = pool.tile([P, F], F32, tag=tag, name=f"vps{mc}")
                for cc in range(CCH):
                    nc.tensor.matmul(
                        ps[:, :],
                        c_sb[:, cc, mc * P : (mc + 1) * P],
                        wv_sb[:, cc, :],
                        start=(cc == 0),
                        stop=(cc == CCH - 1),
                    )
                nc.vector.tensor_copy(
                    vte_sb[:, mc, :, 0:DH],
                    ps.rearrange("p (h d) -> p h d", d=DH),
                )

            def emit_qk_group(which, oc, ih, pool=None, tag="mx"):
                dst, wt, bias_t, src_sb = (
                    (q_sb, wq_sb, bq_sb, x_sb) if which == "q" else (k_sb, wk_sb, bk_sb, c_sb)
                )
                pool = pool if pool is not None else mxps
                ps = pool.tile([P, F], F32, tag=tag, name=f"{which}ps{oc}{ih}")
                for cc in range(CCH):
                    nc.tensor.matmul(
                        ps[:, :],
                        wt[:, cc, oc * P : (oc + 1) * P],
                        src_sb[:, cc, ih * F : (ih + 1) * F],
                        start=(cc == 0),
                        stop=(cc == CCH - 1),
                    )
                nc.vector.tensor_tensor(
                    dst[:, oc, ih * F : (ih + 1) * F],
                    ps[:, :],
                    bias_t[:, oc : oc + 1].to_broadcast([P, F]),
                    mybir.AluOpType.add,
                )

            def oproj_mms(ps, ic, oc, ccs, start, stop):
                for cc in ccs:
                    nc.tensor.matmul(
                        ps[:, :],
                        wo_sb[:, cc, oc * P : (oc + 1) * P],
                        oallP[cc][:, 4 * ic : 4 * ic + 4, :],
                        start=start and cc == ccs[0],
                        stop=stop and cc == ccs[-1],
                    )

            def oproj_fin(ps, ic, oc, on_act=False, plain=False):
                fin = finp.tile([P, F], F16, tag="fin", name=f"fin{ic}{oc}")
                if on_act:
                    nc.scalar.activation(
                        out=fin[:, :],
                        in_=ps[:, :],
                        func=mybir.ActivationFunctionType.Copy,
                    )
                elif plain:
                    nc.vector.tensor_copy(fin[:, :], ps[:, :])
                else:
                    nc.vector.tensor_tensor(
                        fin[:, :],
                        ps[:, :],
                        bo_sb[:, oc : oc + 1].to_broadcast([P, F]),
                        mybir.AluOpType.add,
                    )
                nc.sync.dma_start(
                    out=out_d[oc * P : (oc + 1) * P, ic * F : (ic + 1) * F],
                    in_=fin[:, :],
                )

            def emit_oproj(ic, oc, pool=None, tag="mx"):
                pool = pool if pool is not None else mxps
                ps = pool.tile([P, F], F32, tag=tag, name=f"ops{ic}{oc}")
                oproj_mms(ps, ic, oc, list(range(CCH)), True, True)
                oproj_fin(ps, ic, oc)

            # ---- attention ----
            # Iterations are keyed (ic, pr, hb): one i-half of one head.
            # Per iteration: 8 sim matmuls [128j, 512i] (K=64) through PSUM
            # groups of 3/3/2 banks, exp'd by ACT into es[j, jc, i] fp16.
            # The PREVIOUS iteration's attn@v runs transposed at the front:
            # 32 matmuls lhsT=es[j, i128] rhs=vte[j, 65] -> ot[128, 4, 65]
            # (one bank, slice-wise accumulation), then DVE normalizes via
            # the ones-column denominator and writes oallT fp16.

            def emit_attn(pic, ppr, phb, pes, tag):
                # ot [128, 4, 65]: slot s = i-chunk within the i-half; one
                # PSUM bank, slice-wise accumulation (one group for the bank)
                ot = otp.tile([P, 4, DH + 1], F32, tag="ot", name=f"ot{tag}")
                n_mm = 0
                for jc in range(JCH):
                    for s in range(4):
                        nc.tensor.matmul(
                            ot[:, s, :],
                            pes[:, jc, s * P : (s + 1) * P],
                            vte_sb[:, jc, 2 * ppr + phb, :],
                            start=(n_mm == 0),
                            stop=(n_mm == 4 * JCH - 1),
                            skip_group_check=True,
                        )
                        n_mm += 1
                rec = sbcp.tile([P, 4, 1], F32, tag="rec", name=f"rec{tag}")
                nc.vector.reciprocal(out=rec[:, :, :], in_=ot[:, :, DH : DH + 1])
                nc.vector.tensor_tensor(
                    oallT[ppr][:, 4 * pic : 4 * pic + 4, phb * DH : (phb + 1) * DH],
                    ot[:, :, 0:DH],
                    rec.to_broadcast([P, 4, DH]),
                    mybir.AluOpType.mult,
                )

            def emit_transpose(pic, ppr, via_pe=False, tp_pool=None, tp_tag="ot"):
                # oallT[pr][:, 4*pic:4*pic+4, :] viewed [128, 512] ->
                # oallP[pr][hd, slot, i_lo]  (xbar: out[p,e,l] = in[l, e*128+p]).
                # The DMA xbar path is free engine-wise but has ~3us latency;
                # the tail uses the PE (idle by then) + one DVE copy instead.
                if not via_pe:
                    nc.sync.dma_start_transpose(
                        out=oallP[ppr][:, 4 * pic : 4 * pic + 4, :],
                        in_=oallT[ppr][:, 4 * pic : 4 * pic + 4, :],
                    )
                    return
                pool_ = tp_pool if tp_pool is not None else otp
                tp = pool_.tile([P, 4, P], F16, tag=tp_tag, name=f"tp{pic}{ppr}")
                for s in range(4):
                    nc.tensor.matmul(
                        tp[:, s, :],
                        oallT[ppr][:, 4 * pic + s, :],
                        ident_sb[:, :],
                        is_transpose=True,
                        start=(s == 0),
                        stop=(s == 3),
                        skip_group_check=True,
                    )
                nc.vector.tensor_copy(
                    oallP[ppr][:, 4 * pic : 4 * pic + 4, :], tp[:, :, :]
                )

            iters = [
                (ic, pr, hb) for ic in range(ICH) for pr in range(CCH) for hb in range(2)
            ]
            NIT = len(iters)

            # Pre-loop: just the first pair's q/k (across the three free
            # psum pools) so the sim/exp pipeline starts as early as the
            # DMAs allow. Everything else streams into the iterations.
            emit_qk_group("q", 0, 0, pool=simA, tag="sg")
            emit_qk_group("k", 0, 0, pool=simB, tag="sg")

            # Filler jobs per iteration, split into "mid" (between sim
            # groups g1 and g2 — fills the window where g2's psum slot
            # waits on this iteration's own g0 exp) and "post" (after g2).
            def QK(which, o, ih):
                return lambda p, t: emit_qk_group(which, o, ih, pool=p, tag=t)

            def VG(m):
                return lambda p, t: emit_v_group(m, pool=p, tag=t)

            def OP(ic_, oc_):
                return lambda p, t: emit_oproj(ic_, oc_, pool=mxps, tag="mx")

            def WAVE(px, last=False):
                return ("wave", px, last)

            pre_g1_jobs: dict[int, list] = {i: [] for i in range(NIT)}
            mid_jobs: dict[int, list] = {i: [] for i in range(NIT)}
            post_jobs: dict[int, list] = {i: [] for i in range(NIT)}
            pre_g1_jobs[0] += [QK("k", 0, 1)]
            post_jobs[0] += [QK("q", 1, 0), QK("k", 1, 0)]
            post_jobs[1] += [QK("k", 1, 1), QK("q", 2, 0), QK("k", 2, 0)]
            post_jobs[2] += [QK("k", 2, 1), QK("q", 3, 0), VG(0), VG(1)]
            post_jobs[3] += [QK("k", 3, 0), QK("k", 3, 1), VG(2), VG(3)]
            post_jobs[4] += [VG(4), VG(5), VG(6), VG(7)]
            for pr_ in range(CCH):
                post_jobs[6 + pr_].append(QK("q", pr_, 1))
            for oc_ in range(CCH):
                post_jobs[11 + oc_].append(OP(0, oc_))

            # attention waves: DMA-paced ramp-in means vte is only complete
            # around iteration 4, so waves start there (es bufs=5 covers the
            # lag); wave 14 in iter 15's post slot, wave 15 chased at the tail
            wave_mid = {t: [t - 3] for t in range(7, 16)}
            wave_mid[5] = [0, 1]
            wave_mid[6] = [2, 3]

            GROUPS = ((0, 2), (2, 4), (4, 6), (6, 8))  # jc ranges per group
            SIMPOOLS = (simA, simB, simC)

            es_of = {}
            gctr = [0]  # global sim-group counter -> pool rotation of period 3
            prot = [0]  # rotating pool index for streamed projection groups
            PROT = ((mxps, "mx"), (simA, "sg"), (simB, "sg"), (simC, "sg"))

            def run_proj(job):
                pool, tag = PROT[prot[0] % 3]
                prot[0] += 1
                job(pool, tag)

            def emit_attn_wave(px, last=False):
                pic, ppr, phb = iters[px]
                emit_attn(pic, ppr, phb, es_of[px], tag=f"a{px}")
                if phb == 1:
                    # ic=1 pairs (1,1)/(1,2) transpose on the PE (low latency;
                    # the xbar-DMA path takes ~3us and would stall the tail's
                    # pre-opened o-proj groups)
                    if pic == 1 and ppr in (1, 2):
                        emit_transpose(pic, ppr, via_pe=True)
                    else:
                        emit_transpose(pic, ppr, via_pe=last)

            for it_idx, (ic, pr, hb) in enumerate(iters):
                es = expp.tile([P, JCH, F], F16, tag="es", name=f"es{ic}{pr}{hb}")
                es_of[it_idx] = es

                def sim_group(gi):
                    j0, j1 = GROUPS[gi]
                    pool = SIMPOOLS[gctr[0] % 3]
                    gctr[0] += 1
                    g = pool.tile([P, 2, F], F32, tag="sg", name=f"g{it_idx}{gi}")
                    nb = j1 - j0
                    for idx, jc in enumerate(range(j0, j1)):
                        nc.tensor.matmul(
                            g[:, idx, :],
                            k_sb[hb * DH : (hb + 1) * DH, pr, jc * P : (jc + 1) * P],
                            q_sb[hb * DH : (hb + 1) * DH, pr, ic * F : (ic + 1) * F],
                            start=True,
                            stop=True,
                        )
                    nc.scalar.activation(
                        out=es[:, j0:j1, :],
                        in_=g[:, 0:nb, :],
                        func=mybir.ActivationFunctionType.Exp,
                    )

                if it_idx == 8:
                    # tail-only constants, loaded mid-kernel off the critical path
                    nc.scalar.dma_start(out=borow_sb[:, :], in_=borow_d[:, :])
                    nc.vector.memset(ones_sb[:, :], 1.0)
                sim_group(0)
                for job in pre_g1_jobs[it_idx]:
                    run_proj(job)
                sim_group(1)
                for wv_ in wave_mid.get(it_idx, []):
                    emit_attn_wave(wv_)
                sim_group(2)
                sim_group(3)
                if it_idx == NIT - 1:
                    emit_attn_wave(NIT - 3)
                    emit_attn_wave(NIT - 2)
                for job in post_jobs[it_idx]:
                    run_proj(job)

            # ---- tail ----
            # Final o-proj groups pre-open: cc 0..2 matmuls run while ACT
            # finishes the last exps; only the cc=3 matmuls wait for the
            # last pair's transpose.
            tail_ps = []
            for oc, (pool, tag) in enumerate(
                ((mxps, "mx"), (simA, "sg"), (simB, "sg"), (simC, "sg"))
            ):
                ps = pool.tile([P, F], F32, tag=tag, name=f"ops1{oc}")
                tail_ps.append(ps)
                # seed the bias via a rank-1 matmul so the fins are plain copies
                nc.tensor.matmul(
                    ps[:, :],
                    borow_sb[0:1, oc * P : (oc + 1) * P],
                    ones_sb[0:1, :],
                    start=True,
                    stop=False,
                )
                oproj_mms(ps, 1, oc, [0, 1, 2], False, False)

            # last wave (1,3,hb=1), chase-split: each jc range fires right
            # after its own exp group lands. The rest of the tail is split
            # into slot-halves A (i 512-767) / B (i 768-1023) so the final
            # stores start as early as possible on the serialized DMA fleet.
            lic, lpr, lhb = iters[NIT - 1]
            les = es_of[NIT - 1]
            lot = otp.tile([P, 4, DH + 1], F32, tag="ot", name="otlast")
            n_mm = 0
            for jc in range(JCH):
                for s in range(4):
                    nc.tensor.matmul(
                        lot[:, s, :],
                        les[:, jc, s * P : (s + 1) * P],
                        vte_sb[:, jc, 2 * lpr + lhb, :],
                        start=(n_mm == 0),
                        stop=(n_mm == 4 * JCH - 1),
                        skip_group_check=True,
                    )
                    n_mm += 1
            lrec = sbcp.tile([P, 4, 1], F32, tag="rec", name="reclast")
            nc.vector.reciprocal(out=lrec[:, :, :], in_=lot[:, :, DH : DH + 1])
            nc.vector.tensor_tensor(
                oallT[lpr][:, 4 * lic : 4 * lic + 4, lhb * DH : (lhb + 1) * DH],
                lot[:, :, 0:DH],
                lrec.to_broadcast([P, 4, DH]),
                mybir.AluOpType.mult,
            )
            emit_transpose(lic, lpr, via_pe=True)
            for oc in range(CCH):
                oproj_mms(tail_ps[oc], 1, oc, [3], False, True)
            for oc in range(CCH):
                oproj_fin(tail_ps[oc], 1, oc, on_act=(oc % 2 == 0), plain=True)

    nc.compile()
    return nc


def prep_inputs(x, context, Wq, bq, Wk, bk, Wv, bv, Wo, bo):
    """Host-side sharding + layout prep. Returns per-core input maps.

    Exact bias folding: bv commutes through the attention average
    (attn weights sum to 1), so final = Wo@(attn_out + bv) + bo
    = Wo@attn_out + (Wo@bv + bo)."""
    xb = np.asarray(x, np.float32).reshape(B, C, NTOK).astype(NPF16)
    cb = np.asarray(context, np.float32).reshape(B, C, NTOK).astype(NPF16)
    wqt = np.ascontiguousarray((np.asarray(Wq, np.float32) * SCALE).T).astype(NPF16)
    wkt = np.ascontiguousarray(np.asarray(Wk, np.float32).T).astype(NPF16)
    wvt = np.ascontiguousarray(np.asarray(Wv, np.float32).T).astype(NPF16)
    wot = np.ascontiguousarray(np.asarray(Wo, np.float32).T).astype(NPF16)
    bqs = (np.asarray(bq, np.float32) * SCALE).astype(np.float32)
    bkf = np.asarray(bk, np.float32)
    bof = (
        np.asarray(Wo, np.float32) @ np.asarray(bv, np.float32)
        + np.asarray(bo, np.float32)
    ).astype(np.float32)
    # bias_pack[p, oc*3 + t]: t=0 bq, 1 bk, 2 bo; channel = oc*128 + p
    bias_pack = np.empty((P, CCH * 3), np.float32)
    for oc in range(CCH):
        bias_pack[:, oc * 3 + 0] = bqs[oc * P : (oc + 1) * P]
        bias_pack[:, oc * 3 + 1] = bkf[oc * P : (oc + 1) * P]
        bias_pack[:, oc * 3 + 2] = bof[oc * P : (oc + 1) * P]
    ident = np.eye(P, dtype=NPF16)
    bo_row = np.ascontiguousarray(bof.reshape(1, C)).astype(NPF16)
    in_maps = []
    for b in range(B):
        in_maps.append(
            {
                "x": np.ascontiguousarray(xb[b]),
                "ctx": np.ascontiguousarray(cb[b]),
                "wqt": wqt,
                "wkt": wkt,
                "wvt": wvt,
                "wot": wot,
                "bias_pack": bias_pack,
                "ident": ident,
                "bo_row": bo_row,
            }
        )
    return in_maps


_NC = None


def _get_nc():
    global _NC
    if _NC is None:
        _NC = build_nc()
    return _NC


def kernel(x, context, Wq, bq, Wk, bk, Wv, bv, Wo, bo):
    from concourse.bass_utils import run_bass_kernel_spmd

    nc = _get_nc()
    in_maps = prep_inputs(x, context, Wq, bq, Wk, bk, Wv, bv, Wo, bo)
    br = run_bass_kernel_spmd(nc, in_maps, list(range(B)))
    out = np.stack([np.asarray(br.results[b]["out"], np.float32) for b in range(B)])
    return out.reshape(B, C, 32, 32)
